# revision 2
# baseline (speedup 1.0000x reference)
"""Trainium2 Bass kernel for the semantic-weighted contrastive loss (v3).

Sharding: data-parallel over B across 8 cores; text replicated.  See v2
docstring for the core transposed-logits / fp8-DoubleRow / one-ACT-table
design.  v3 changes (driven by the v2 trace: gpsimd TT 1.5us/tile, DVE
2-input ops ~1us/[128,512], PE active 94us):

  - Positive-pair term folded into the staged weights:
    W = (sem-1) - sem*onehot(pos), staged transposed fp8.  Then
    denom[b] = -sum_c W[c,b]*ex[c,b] directly -- no sempos/pterm/expp.
  - text^2 staged bf16 (replaces text bf16; same bytes): text norms become
    1-input DVE tensor_reduce (~2x cheaper than 2-input stt squares).
  - Wide tiles: one 2-bank psum [128,2,512] per ct, ONE wide ACT exp and
    ONE wide lane mult per ct (halves per-instruction overheads).
  - ex and prod in fp8 (sim rel err 8e-4 vs 2e-2 gate).
  - ct-accumulation of prod moved OFF the lanes onto PE: pairs of prod
    tiles are summed and accumulated in psum by DoubleRow matmuls against
    a stacked [I;I] fp8 identity (16 mms total), then partition-reduced
    by 8 ones-matmuls into psum[b%128, b//128] for the final loss.
"""

import math
import sys

for _p in ("/opt/trn_rl_repo", "/root/.axon_site/_ro/trn_rl_repo"):
    if _p not in sys.path:
        sys.path.append(_p)

import numpy as np
import ml_dtypes

import concourse.bass as bass
import concourse.mybir as mybir
import concourse.tile as tile
from concourse.bass_utils import run_bass_kernel_spmd
from concourse.masks import make_identity

F32 = mybir.dt.float32
BF16 = mybir.dt.bfloat16
FP8 = mybir.dt.float8e4
AF = mybir.ActivationFunctionType
ALU = mybir.AluOpType
DR = mybir.MatmulPerfMode.DoubleRow

B, C, D = 8192, 4096, 1024
TEMPERATURE = 0.07
INV_T = 1.0 / TEMPERATURE
NCORES = 8
BL = B // NCORES
P = 128
KT = D // P
NBT = BL // P
CT = C // P
SCALE_A = 64.0
SCALE_T = 4.0
BIAS_T = math.log(INV_T / (SCALE_A * SCALE_T))
BIAS_A = math.log(SCALE_A)
POS_SCALE = INV_T / SCALE_A
TSQ_AHEAD = 6

# how many of the 32 wide prod-mults DVE owns (rest gpsimd)
PROD_DVE = 20


def _build_nc() -> bass.Bass:
    nc = bass.Bass()
    a16 = nc.declare_dram_parameter("a16", [BL, D], BF16, isOutput=False)
    tsq16 = nc.declare_dram_parameter("tsq16", [C, D], BF16, isOutput=False)
    tT8 = nc.declare_dram_parameter("tT8", [P, CT, KT, P], FP8, isOutput=False)
    wT8 = nc.declare_dram_parameter("wT8", [C, 2, 512], FP8, isOutput=False)
    tpos16 = nc.declare_dram_parameter("tpos16", [BL, D], BF16, isOutput=False)
    loss = nc.declare_dram_parameter("loss", [P, NBT], F32, isOutput=True)

    orig_sem_clear = type(nc.gpsimd).sem_clear
    type(nc.gpsimd).sem_clear = lambda self, sem: None
    try:
        with tile.TileContext(nc) as tc:
            _body(tc, a16, tsq16, tT8, wT8, tpos16, loss)
    finally:
        type(nc.gpsimd).sem_clear = orig_sem_clear
    mybir.codegen_inst_isa_subclasses(nc)
    _split_waits(nc)
    nc.finalize()
    return nc


def _split_waits(nc):
    """This walrus allows only ONE sync-wait per TPB instruction; hoist
    extras into standalone same-engine EventSemaphore waits."""
    n_new = 0
    for fn in nc.m.functions:
        for bb in fn.blocks:
            new_list = []
            for inst in bb.instructions:
                si = getattr(inst, "sync_info", None)
                if si and si.on_wait and len(si.on_wait) > 1:
                    extra, keep = si.on_wait[:-1], si.on_wait[-1:]
                    for w in extra:
                        n_new += 1
                        wi = mybir.InstEventSemaphore(
                            name=f"{inst.name}_w{n_new}",
                            engine=inst.engine,
                            ins=[],
                            outs=[],
                            sync_info=mybir.SyncInfo(on_wait=[w], on_update=[]),
                        )
                        nc.inst_map[wi.name] = wi
                        new_list.append(wi)
                    si.on_wait = keep
                new_list.append(inst)
            bb.instructions[:] = new_list


def _body(tc, a16, tsq16, tT8, wT8, tpos16, loss):
    nc = tc.nc
    from contextlib import ExitStack

    with ExitStack() as ctx:
        res = ctx.enter_context(tc.tile_pool(name="res", bufs=1))
        tsqpool = ctx.enter_context(tc.tile_pool(name="tsqp", bufs=8))
        tT8pool = ctx.enter_context(tc.tile_pool(name="tT8p", bufs=8))
        wpool = ctx.enter_context(tc.tile_pool(name="wp", bufs=8))
        tppool = ctx.enter_context(tc.tile_pool(name="tpp", bufs=8))
        a8pool = ctx.enter_context(tc.tile_pool(name="a8p", bufs=2))
        dpool = ctx.enter_context(tc.tile_pool(name="dump", bufs=4))
        expool = ctx.enter_context(tc.tile_pool(name="exp", bufs=4))
        prpool = ctx.enter_context(tc.tile_pool(name="prod", bufs=4))
        npool = ctx.enter_context(tc.tile_pool(name="nrm", bufs=8))
        pt = ctx.enter_context(tc.tile_pool(name="ptr", bufs=2, space="PSUM"))
        pm = ctx.enter_context(tc.tile_pool(name="pmm", bufs=2, space="PSUM"))
        pSa = ctx.enter_context(tc.tile_pool(name="pSa", bufs=2, space="PSUM"))

        # resident tensors
        a16r = res.tile([P, NBT, D], BF16, tag="a16r")
        aT = res.tile([P, KT, BL], FP8, tag="aT")
        ssqa = res.tile([P, NBT], F32, tag="ssqa")
        ssqt = res.tile([P, CT], F32, tag="ssqt")
        ssqp = res.tile([P, NBT], F32, tag="ssqp")
        scale_t = res.tile([P, CT], F32, tag="scale_t")
        invA = res.tile([P, NBT], F32, tag="invA")
        invTP = res.tile([P, NBT], F32, tag="invTP")
        plog = res.tile([P, NBT], F32, tag="plog")
        loss_sb = res.tile([P, NBT], F32, tag="loss_sb")
        ident = res.tile([P, P], BF16, tag="ident")
        ident8d = res.tile([P, 2, P], FP8, tag="ident8d")
        ones = res.tile([P, 1], F32, tag="ones")
        bias_a = res.tile([P, 1], F32, tag="bias_a")
        bias_t = res.tile([P, 1], F32, tag="bias_t")
        bias_z = res.tile([P, 1], F32, tag="bias_z")
        accS = [res.tile([P, 512], F32, tag=f"accS{h}", name=f"accS{h}")
                for h in range(2)]

        make_identity(nc, ident[:])
        make_identity(nc, ident8d[:, 0, :])
        make_identity(nc, ident8d[:, 1, :])
        nc.vector.memset(ones[:], 1.0)
        nc.vector.memset(bias_a[:], BIAS_A)
        nc.vector.memset(bias_t[:], BIAS_T)
        nc.vector.memset(bias_z[:], 0.0)

        # psum accumulators for the prod partial sums (one per b-half)
        psum_acc = [pSa.tile([P, 512], F32, tag="pSa", name=f"pSa{h}")
                    for h in range(2)]

        def ssq_stt(x_ap, accum_ap, tag):
            dmp = dpool.tile([P, D], BF16, tag="dump", name=f"d_{tag}")
            nc.vector.scalar_tensor_tensor(
                out=dmp[:], in0=x_ap, scalar=0.0, in1=x_ap,
                op0=ALU.bypass, op1=ALU.mult, accum_out=accum_ap,
            )

        def ssq_act(x_ap, accum_ap, tag):
            dmp = dpool.tile([P, D], BF16, tag="dump", name=f"d_{tag}")
            nc.scalar.activation(dmp[:], x_ap, AF.Square, accum_out=accum_ap)

        def rsqrt_batch(out_ap, ssq_ap, bias_ap, tag, n):
            ls = npool.tile([P, n], F32, tag="ls", name=f"ls_{tag}")
            nc.scalar.activation(ls[:], ssq_ap, AF.Ln)
            nc.scalar.activation(out_ap, ls[:], AF.Exp, scale=-0.5, bias=bias_ap)

        def text_norm_group(g):
            for i in range(4):
                ct = 4 * g + i
                tq = tsqpool.tile([P, D], BF16, tag="tsq", name=f"tsq_{ct}")
                nc.sync.dma_start(tq[:], tsq16[ct * P : (ct + 1) * P, :])
                if i == 3:
                    # ACT Copy+accum sums the (already squared) tile
                    dmp = dpool.tile([P, D], BF16, tag="dump", name=f"dq_{ct}")
                    nc.scalar.activation(
                        dmp[:], tq[:], AF.Copy, accum_out=ssqt[:, ct : ct + 1]
                    )
                else:
                    nc.vector.tensor_reduce(
                        out=ssqt[:, ct : ct + 1], in_=tq[:],
                        axis=mybir.AxisListType.X, op=ALU.add,
                    )
            sl = slice(4 * g, 4 * g + 4)
            rsqrt_batch(scale_t[:, sl], ssqt[:, sl], bias_t[:], f"tg{g}", 4)

        # ---- audio phase ----
        for bt in range(NBT):
            nc.sync.dma_start(a16r[:, bt, :], a16[bt * P : (bt + 1) * P, :])
            if bt % 2 == 0:
                ssq_stt(a16r[:, bt, :], ssqa[:, bt : bt + 1], f"a{bt}")
            else:
                ssq_act(a16r[:, bt, :], ssqa[:, bt : bt + 1], f"a{bt}")
        rsqrt_batch(invA[:], ssqa[:], bias_a[:], "ab", NBT)
        for bt in range(NBT):
            a8 = a8pool.tile([P, D], BF16, tag="a8", name=f"a8_{bt}")
            nc.vector.tensor_scalar_mul(
                out=a8[:], in0=a16r[:, bt, :], scalar1=invA[:, bt : bt + 1]
            )
            ps = pt.tile([P, KT, P], BF16, tag="ptr", name=f"ptr_{bt}")
            for j in range(KT):
                nc.tensor.transpose(ps[:, j, :], a8[:, j * P : (j + 1) * P], ident[:])
            nc.scalar.activation(aT[:, :, bt * P : (bt + 1) * P], ps[:], AF.Copy)

        text_norm_group(0)
        text_norm_group(1)

        # ---- text rounds ----
        pairs = {}   # pair index -> prod pair tile
        n_emitted = [0]

        def emit_pair_accum(p_idx, is_last):
            pp = pairs.pop(p_idx)
            for bh in range(2):
                nc.tensor.matmul(
                    psum_acc[bh][:],
                    lhsT=ident8d[:],
                    rhs=pp[:, :, bh, :],
                    start=(p_idx == 0),
                    stop=is_last,
                    perf_mode=DR,
                    skip_group_check=True,
                )
            n_emitted[0] += 1

        for ct in range(CT):
            tt8 = tT8pool.tile([P, KT, P], FP8, tag="tt8", name=f"tt8_{ct}")
            nc.sync.dma_start(tt8[:], tT8[:, ct, :, :])
            wt8 = wpool.tile([P, 2, 512], FP8, tag="wt8", name=f"wt8_{ct}")
            nc.sync.dma_start(wt8[:], wT8[ct * P : (ct + 1) * P, :, :])

            if (ct + 2) % 4 == 0 and 2 <= (ct + 2) // 4 <= 7:
                text_norm_group((ct + 2) // 4)

            pmt = pm.tile([P, 2, 512], F32, tag="pmm", name=f"pm_{ct}")
            for kp in range(4):
                for bh in range(2):
                    nc.tensor.matmul(
                        pmt[:, bh, :],
                        lhsT=tt8[:, 2 * kp : 2 * kp + 2, :],
                        rhs=aT[:, 2 * kp : 2 * kp + 2, bh * 512 : (bh + 1) * 512],
                        start=(kp == 0),
                        stop=(kp == 3),
                        perf_mode=DR,
                        skip_group_check=True,
                    )
            ex = expool.tile([P, 2, 512], FP8, tag="ex", name=f"ex_{ct}")
            nc.scalar.activation(ex[:], pmt[:], AF.Exp, scale=scale_t[:, ct : ct + 1])

            if ct % 2 == 0:
                pairs[ct // 2] = prpool.tile(
                    [P, 2, 2, 512], FP8, tag="pp", name=f"pp_{ct // 2}"
                )
            pp = pairs[ct // 2]
            eng = nc.vector if ct % 8 < (PROD_DVE // 4) else nc.gpsimd
            eng.tensor_tensor(pp[:, ct % 2, :, :], wt8[:], ex[:], ALU.mult)

            # pair-accumulate on PE with a 2-round lag (PE never waits lanes)
            if ct % 2 == 1 and ct >= 3:
                emit_pair_accum((ct - 3) // 2, is_last=False)

            if ct == 14:
                _pos_phase_a(nc, tppool, res, ssqp, a16r, tpos16,
                             ssq_stt, ssq_act)
            if ct == 22:
                _pos_phase_b(nc, dpool, npool, a16r, invA, invTP, ssqp,
                             plog, rsqrt_batch, bias_z, tc)

        emit_pair_accum(15, is_last=True)

        # ---- partition-reduce: psum_acc -> SBUF -> 8 ones-matmuls ----
        for h in range(2):
            nc.scalar.activation(accS[h][:], psum_acc[h][:], AF.Copy)
        psS = pm.tile([P, NBT], F32, tag="pmm", name="psS")
        for bs in range(NBT):
            h, col = (0, bs) if bs < 4 else (1, bs - 4)
            nc.tensor.matmul(
                psS[:, bs : bs + 1],
                lhsT=accS[h][:, col * P : (col + 1) * P],
                rhs=ones[:],
                start=True,
                stop=True,
            )

        # ---- loss = ln(-S) - plog ----
        lnv = npool.tile([P, NBT], F32, tag="lnv")
        nc.scalar.activation(lnv[:], psS[:], AF.Ln, scale=-1.0)
        nc.vector.tensor_tensor(loss_sb[:], lnv[:], plog[:], ALU.subtract)
        nc.sync.dma_start(loss[:], loss_sb[:])


_TPOS_TILES = []


def _pos_phase_a(nc, tppool, res, ssqp, a16r, tpos16, ssq_stt, ssq_act):
    _TPOS_TILES.clear()
    for bt in range(NBT):
        tp = tppool.tile([P, D], BF16, tag="tp", name=f"tp_{bt}")
        nc.sync.dma_start(tp[:], tpos16[bt * P : (bt + 1) * P, :])
        if bt % 2 == 0:
            ssq_act(tp[:], ssqp[:, bt : bt + 1], f"p{bt}")
        else:
            ssq_stt(tp[:], ssqp[:, bt : bt + 1], f"p{bt}")
        _TPOS_TILES.append(tp)


def _pos_phase_b(nc, dpool, npool, a16r, invA, invTP, ssqp, plog,
                 rsqrt_batch, bias_z, tc):
    rsqrt_batch(invTP[:], ssqp[:], bias_z[:], "pb", NBT)
    for bt in range(NBT):
        rdot = npool.tile([P, 1], F32, tag="rdot", name=f"rdot_{bt}")
        dmp = dpool.tile([P, D], BF16, tag="dump", name=f"dp_{bt}")
        nc.vector.scalar_tensor_tensor(
            out=dmp[:], in0=a16r[:, bt, :], scalar=0.0, in1=_TPOS_TILES[bt][:],
            op0=ALU.bypass, op1=ALU.mult, accum_out=rdot[:],
        )
        inv2 = npool.tile([P, 1], F32, tag="inv2", name=f"inv2_{bt}")
        nc.vector.tensor_tensor(
            inv2[:], invA[:, bt : bt + 1], invTP[:, bt : bt + 1], ALU.mult
        )
        pd = npool.tile([P, 1], F32, tag="pd", name=f"pd_{bt}")
        nc.vector.tensor_tensor(pd[:], rdot[:], inv2[:], ALU.mult)
        nc.scalar.activation(plog[:, bt : bt + 1], pd[:], AF.Copy, scale=POS_SCALE)
    _TPOS_TILES.clear()


_NC_CACHE = None


def _get_nc() -> bass.Bass:
    global _NC_CACHE
    if _NC_CACHE is None:
        _NC_CACHE = _build_nc()
    return _NC_CACHE


def make_in_maps(audio_embeddings, text_embeddings, semantic_weights, pos_idx):
    audio_embeddings = np.asarray(audio_embeddings, dtype=np.float32)
    text_embeddings = np.asarray(text_embeddings, dtype=np.float32)
    semantic_weights = np.asarray(semantic_weights, dtype=np.float32)
    pos_idx = np.asarray(pos_idx, dtype=np.int32)

    bf16 = ml_dtypes.bfloat16
    fp8 = ml_dtypes.float8_e4m3

    tsq_h = np.ascontiguousarray(
        (text_embeddings * text_embeddings).astype(bf16)
    )
    tT8_h = np.ascontiguousarray(
        (text_embeddings.T * np.float32(SCALE_T))
        .astype(fp8)
        .reshape(KT, P, CT, P)
        .transpose(1, 2, 0, 3)
    )

    in_maps = []
    for k in range(NCORES):
        sl = slice(k * BL, (k + 1) * BL)
        pos_k = pos_idx[sl]
        w = semantic_weights[sl] - 1.0
        w[np.arange(BL), pos_k] = -1.0
        in_maps.append(
            {
                "a16": np.ascontiguousarray(audio_embeddings[sl].astype(bf16)),
                "tsq16": tsq_h,
                "tT8": tT8_h,
                "wT8": np.ascontiguousarray(w.T.astype(fp8).reshape(C, 2, 512)),
                "tpos16": np.ascontiguousarray(
                    text_embeddings[pos_k].astype(bf16)
                ),
            }
        )
    return in_maps


def run_sharded(inputs: dict, trace: bool = False):
    nc = _get_nc()
    in_maps = make_in_maps(**inputs)
    res = run_bass_kernel_spmd(
        nc, in_maps, list(range(NCORES)), trace=trace,
        trace_cores=[0] if trace else None,
    )
    rows = np.concatenate([r["loss"].T.reshape(BL) for r in res.results])
    val = np.float32(rows.mean(dtype=np.float64))
    return val, res


def kernel(**inputs) -> np.ndarray:
    val, _ = run_sharded(inputs, trace=False)
    return np.asarray(val, dtype=np.float32)
